# revision 2
# baseline (speedup 1.0000x reference)
"""CRF forward/backward (alpha/beta) recurrence kernel for Trainium2, 8 NeuronCores.

Strategy:
  - Host precomputes expT = exp(T), expTT = exp(T).T and E = exp(scores) in fp32.
  - Class dim (4096) is tensor-parallel across 8 cores: core c owns columns
    [c*512, (c+1)*512) of both recurrences.
  - Per step, the state vector (alpha or beta, 4096 wide) is the *stationary*
    matmul operand (lhsT = [128, 1] per k-tile; loading 1 weight column is
    nearly free) and the transition-matrix slice streams through as rhs
    [128, 512]:
        psum[1, 512] += state[:, k].T @ W[k-tile, :]   (32 accumulating matmuls)
    This keeps the PE's rhs-streaming bus (the fast path) saturated instead of
    paying the 128-cycle stationary-weight load per tile.
  - The per-core 512-wide result slice is multiplied by exp(scores[i, slice]),
    written to that core's output slice, and AllGather'd (2 KB/rank) so every
    core has the full next-state vector.  The fwd and bwd chains interleave on
    the PE so each chain's gather latency hides under the other chain's
    matmuls.
"""

import numpy as np

SENT_LEN = 2048
CLASS_NUM = 4096
N_CORES = 8
SLICE = CLASS_NUM // N_CORES  # 512
KT = CLASS_NUM // 128  # 32 k-tiles

_NC_CACHE = {}
_RUNNER_CACHE = {}


def _build(n_steps, w_dtype_name="float32"):
    """Build the Bass module. n_steps = number of recurrence steps per chain
    (SENT_LEN - 1 for the real problem)."""
    import concourse.bacc as bacc
    import concourse.tile as tile
    import concourse.mybir as mybir

    fp32 = mybir.dt.float32
    wdt = getattr(mybir.dt, w_dtype_name)

    nc = bacc.Bacc("TRN2", target_bir_lowering=False, debug=False,
                   num_devices=N_CORES)

    L = n_steps + 1
    # Per-core inputs
    wf = nc.dram_tensor("wf", [CLASS_NUM, SLICE], wdt, kind="ExternalInput")
    wb = nc.dram_tensor("wb", [CLASS_NUM, SLICE], wdt, kind="ExternalInput")
    es = nc.dram_tensor("es", [L, SLICE], fp32, kind="ExternalInput")
    a0 = nc.dram_tensor("a0", [128, KT], fp32, kind="ExternalInput")
    bL = nc.dram_tensor("bL", [128, KT], fp32, kind="ExternalInput")
    # Per-core outputs (rows 1..L-1 of alpha, rows 0..L-2 of beta are written)
    oa = nc.dram_tensor("oa", [L, SLICE], fp32, kind="ExternalOutput")
    ob = nc.dram_tensor("ob", [L, SLICE], fp32, kind="ExternalOutput")

    rg = [list(range(N_CORES))]

    with tile.TileContext(nc) as tc:
        with (
            tc.tile_pool(name="w", bufs=1) as wpool,
            tc.tile_pool(name="state", bufs=2) as spool,
            tc.tile_pool(name="ps", bufs=2, space="PSUM") as pspool,
            tc.tile_pool(name="sb", bufs=3) as sbpool,
            tc.tile_pool(name="ein", bufs=6) as epool,
            tc.tile_pool(name="dram", bufs=3, space="DRAM") as dpool,
        ):
            # Transition matrix slices, [128, KT*SLICE]: k-tile k in columns
            # [k*SLICE, (k+1)*SLICE)
            wf_sb = wpool.tile([128, KT * SLICE], wdt, name="wf_sb")
            wb_sb = wpool.tile([128, KT * SLICE], wdt, name="wb_sb")
            for k in range(KT):
                nc.sync.dma_start(wf_sb[:, k * SLICE:(k + 1) * SLICE],
                                  wf[k * 128:(k + 1) * 128, :])
                nc.sync.dma_start(wb_sb[:, k * SLICE:(k + 1) * SLICE],
                                  wb[k * 128:(k + 1) * 128, :])

            # chain ids: 0 = fwd (alpha), 1 = bwd (beta)
            state = [None, None]
            state[0] = spool.tile([128, KT], fp32, name="st_f", tag="st_f")
            state[1] = spool.tile([128, KT], fp32, name="st_b", tag="st_b")
            nc.sync.dma_start(state[0][:], a0[:])
            nc.sync.dma_start(state[1][:], bL[:])

            w_sb = [wf_sb, wb_sb]
            out_d = [oa, ob]

            for t in range(1, n_steps + 1):
                for ch in range(2):
                    row = t if ch == 0 else L - 1 - t
                    ps = pspool.tile([1, SLICE], fp32, name="ps",
                                     tag=f"ps{ch}")
                    st = state[ch]
                    for k in range(KT):
                        nc.tensor.matmul(
                            ps[:, :],
                            st[:, k:k + 1],
                            w_sb[ch][:, k * SLICE:(k + 1) * SLICE],
                            start=(k == 0),
                            stop=(k == KT - 1),
                        )
                    e_t = epool.tile([1, SLICE], fp32, name="e_t",
                                     tag=f"e{ch}")
                    nc.sync.dma_start(e_t[:], es[row:row + 1, :])
                    a_sb = sbpool.tile([1, SLICE], fp32, name="a_sb",
                                       tag=f"a{ch}")
                    nc.vector.tensor_mul(a_sb[:], ps[:, :], e_t[:])
                    nc.sync.dma_start(out_d[ch][row:row + 1, :], a_sb[:])

                    if t < n_steps:
                        g_in = dpool.tile([1, SLICE], fp32, name="g_in",
                                          tag=f"gi{ch}")
                        g_out = dpool.tile([N_CORES, SLICE], fp32,
                                           name="g_out", tag=f"go{ch}")
                        nc.sync.dma_start(g_in[:], a_sb[:])
                        nc.gpsimd.collective_compute(
                            "AllGather",
                            mybir.AluOpType.bypass,
                            replica_groups=rg,
                            ins=[g_in[:].opt()],
                            outs=[g_out[:].opt()],
                        )
                        nst = spool.tile([128, KT], fp32, name="nst",
                                         tag=f"st_{'fb'[ch]}")
                        nc.sync.dma_start(
                            nst[:],
                            g_out[:].rearrange("r (k p) -> p (r k)", p=128),
                        )
                        state[ch] = nst

    nc.finalize()
    return nc


def _get_nc(n_steps, w_dtype_name="float32"):
    key = (n_steps, w_dtype_name)
    if key not in _NC_CACHE:
        _NC_CACHE[key] = _build(n_steps, w_dtype_name)
    return _NC_CACHE[key]


def _make_runner(nc, n_cores=N_CORES):
    """Compile nc into a reusable jitted callable over device-resident inputs.

    Returns (run, load, fetch): load(in_maps) puts per-core inputs on device;
    run() executes and blocks; fetch(out) returns per-core output dicts.
    """
    import jax
    import concourse.mybir as mybir
    from jax.sharding import Mesh, PartitionSpec, NamedSharding
    from jax.experimental.shard_map import shard_map
    from concourse.bass2jax import (
        _bass_exec_p, install_neuronx_cc_hook, partition_id_tensor,
    )

    install_neuronx_cc_hook()
    partition_name = (nc.partition_id_tensor.name
                      if nc.partition_id_tensor else None)
    in_names, out_names, out_avals, zero_outs = [], [], [], []
    for alloc in nc.m.functions[0].allocations:
        if not isinstance(alloc, mybir.MemoryLocationSet):
            continue
        name = alloc.memorylocations[0].name
        if alloc.kind == "ExternalInput":
            if name != partition_name:
                in_names.append(name)
        elif alloc.kind == "ExternalOutput":
            shape = tuple(alloc.tensor_shape)
            dtype = mybir.dt.np(alloc.dtype)
            out_names.append(name)
            out_avals.append(jax.core.ShapedArray(shape, dtype))
            zero_outs.append(np.zeros(shape, dtype))
    n_params = len(in_names)
    all_in_names = in_names + out_names
    if partition_name is not None:
        all_in_names.append(partition_name)

    def _body(*args):
        operands = list(args)
        if partition_name is not None:
            operands.append(partition_id_tensor())
        outs = _bass_exec_p.bind(
            *operands,
            out_avals=tuple(out_avals),
            in_names=tuple(all_in_names),
            out_names=tuple(out_names),
            lowering_input_output_aliases=(),
            sim_require_finite=True,
            sim_require_nnan=True,
            nc=nc,
        )
        return tuple(outs)

    devices = jax.devices()[:n_cores]
    mesh = Mesh(np.asarray(devices), ("core",))
    in_specs = (PartitionSpec("core"),) * (n_params + len(out_names))
    out_specs = (PartitionSpec("core"),) * len(out_names)
    sharded = jax.jit(
        shard_map(_body, mesh=mesh, in_specs=in_specs, out_specs=out_specs,
                  check_rep=False),
        keep_unused=True,
    )
    sh = NamedSharding(mesh, PartitionSpec("core"))

    def load(in_maps):
        per_core = [[np.asarray(m[name]) for name in in_names]
                    for m in in_maps]
        concat_in = [
            np.concatenate([per_core[c][i] for c in range(n_cores)], axis=0)
            for i in range(n_params)
        ]
        concat_zeros = [
            np.zeros((n_cores * z.shape[0], *z.shape[1:]), z.dtype)
            for z in zero_outs
        ]
        return [jax.device_put(a, sh) for a in concat_in + concat_zeros]

    def run(dev_in):
        out = sharded(*dev_in)
        jax.block_until_ready(out)
        return out

    def fetch(out):
        return [
            {name: np.asarray(out[i]).reshape(n_cores, *out_avals[i].shape)[c]
             for i, name in enumerate(out_names)}
            for c in range(n_cores)
        ]

    return run, load, fetch


def _prep_inputs(scores, T):
    L = scores.shape[0]
    expT = np.exp(T.astype(np.float32))
    expTT = np.ascontiguousarray(expT.T)
    E = np.exp(scores.astype(np.float32))
    a0 = np.ascontiguousarray(E[0].reshape(KT, 128).T)  # [128, KT]
    bL = np.ascontiguousarray(E[L - 1].reshape(KT, 128).T)
    in_maps = []
    for c in range(N_CORES):
        sl = slice(c * SLICE, (c + 1) * SLICE)
        in_maps.append({
            "wf": np.ascontiguousarray(expT[:, sl]),
            "wb": np.ascontiguousarray(expTT[:, sl]),
            "es": np.ascontiguousarray(E[:, sl]),
            "a0": a0,
            "bL": bL,
        })
    return in_maps, E


def get_runner(n_steps, w_dtype_name="float32"):
    key = (n_steps, w_dtype_name)
    if key not in _RUNNER_CACHE:
        nc = _get_nc(n_steps, w_dtype_name)
        _RUNNER_CACHE[key] = _make_runner(nc)
    return _RUNNER_CACHE[key]


def _run(scores, T, n_steps=None):
    L, C = scores.shape
    if n_steps is None:
        n_steps = L - 1
    in_maps, E = _prep_inputs(scores, T)
    run, load, fetch = get_runner(n_steps)
    dev_in = load(in_maps)
    out = run(dev_in)
    results = fetch(out)

    alpha = np.empty((L, C), dtype=np.float32)
    beta = np.empty((L, C), dtype=np.float32)
    for c in range(N_CORES):
        sl = slice(c * SLICE, (c + 1) * SLICE)
        alpha[:, sl] = results[c]["oa"]
        beta[:, sl] = results[c]["ob"]
    alpha[0] = E[0]
    beta[L - 1] = E[L - 1]
    return alpha, beta


def kernel(scores, T):
    scores = np.asarray(scores, dtype=np.float32)
    T = np.asarray(T, dtype=np.float32)
    return _run(scores, T)
